# revision 1
# baseline (speedup 1.0000x reference)
"""GBST (Charformer) layer kernel for Trainium2, 8 NeuronCores, batch-parallel.

Per core (one batch element):
  x (512, 4096) --conv K=5 VALID--> y (512, 4092), computed TRANSPOSED as
  yT tiles [128 l, 512 e] on PSUM (f32r matmuls, x slices as stationary operand).
  Scores s1[l] = score . y[:, l] via DVE tensor_tensor_reduce on yT.
  Multi-width pooling (w=2,3,4) + repeat-expansion fused into single PE matmuls
  with constant block-banded matrices (entries 1/w).  Softmax over w runs in
  column space [128 l, 4] (trivially parallel).  The combine is per-partition
  tensor_scalar/scalar_tensor_tensor ops.  Final avg-pool-by-2 + transpose back
  to [e, m] is one more PE matmul per e-chunk with lhsT = combined tile.
  conv bias is algebraically dropped everywhere (softmax shift-invariance,
  sum_w A = 1) and added once at the final eviction.

float32r (E8M11) is used for all matmuls: inputs are pre-rounded on host /
rounded by the producing engine, giving ~1e-4 end-to-end error at bf16 speed.
"""
import os, sys
sys.path.insert(0, "/opt/trn_rl_repo")
import numpy as np

import concourse.bacc as bacc
import concourse.mybir as mybir
from concourse.tile import TileContext
from concourse.bass_utils import run_bass_kernel_spmd

F32R = mybir.dt.float32r
F32 = mybir.dt.float32
B, E, L = 8, 512, 4096
K = 5
L2 = L - K + 1            # 4092
NT = 32                   # l-tiles of 128 (last has 124 rows)
NCHUNK = 4                # e chunks of 128
MOUT = L // 2             # 2048
ALU = mybir.AluOpType
ACTF = mybir.ActivationFunctionType


def round_fp32r(x: np.ndarray) -> np.ndarray:
    """Round fp32 to fp32r (E8M11: low 12 mantissa bits dropped, RNE)."""
    u = np.ascontiguousarray(x, np.float32).view(np.uint32).astype(np.uint64)
    u = u + 0x7FF + ((u >> 12) & 1)
    return (u & 0xFFFFF000).astype(np.uint32).view(np.float32)


def _pool3_mats():
    """Per-phase main/prev/next lhsT matrices for w=3 pooling-expansion.

    main[ph][l', l] = 1/3 if global blocks match within the tile (ph = 128t mod 3)
    prev[ph][l', l] : contribution of previous tile's row l' (only l' in {126,127})
    next[ph][l', l] : contribution of next tile's row l' (only l' in {0,1})
    """
    m = np.zeros((3, 128, 128), np.float32)
    p = np.zeros((3, 128, 128), np.float32)
    n = np.zeros((3, 128, 128), np.float32)
    for ph in (0, 1, 2):
        t = [3, 2, 1][ph]          # representative t with (128*t) % 3 == ph
        assert (128 * t) % 3 == ph
        base = 128 * t
        for l in range(128):
            blk = (base + l) // 3
            for lp in range(128):
                if (base + lp) // 3 == blk:
                    m[ph, lp, l] = 1.0 / 3.0
                if (base - 128 + lp) // 3 == blk:
                    p[ph, lp, l] = 1.0 / 3.0
                if (base + 128 + lp) // 3 == blk:
                    n[ph, lp, l] = 1.0 / 3.0
    assert np.all(p[:, :126, :] == 0) and np.all(n[:, 2:, :] == 0)
    return m, p, n


def _host_consts(conv_w, conv_b, score_w):
    # wv: [128, 20*512]; block j = 5*c + k holds wT[k, chunk c]: [i, e] = w[e, 128c+i, k]
    wt = conv_w.transpose(1, 0, 2).reshape(NCHUNK, 128, E, K)   # (c, i, e, k)
    wv = wt.transpose(1, 0, 3, 2).reshape(128, NCHUNK * K * E)  # i, (c,k), e
    wv = round_fp32r(wv)

    scb = np.broadcast_to(score_w.astype(np.float32), (128, E)).copy()

    pc2 = np.zeros((128, 128), np.float32)
    pc4 = np.zeros((128, 128), np.float32)
    for l in range(128):
        pc2[(l // 2) * 2:(l // 2) * 2 + 2, l] = 0.5
        pc4[(l // 4) * 4:(l // 4) * 4 + 4, l] = 0.25
    pc3m, pc3p, pc3n = _pool3_mats()
    dsm = np.zeros((128, 64), np.float32)
    for l in range(128):
        dsm[l, l // 2] = 0.5
    # pack: pc2 [0:128], pc4 [128:256], pc3m [256:640], pc3p [640:1024],
    #       pc3n [1024:1408], dsm [1408:1472]
    pcs = np.concatenate(
        [pc2, pc4] + [pc3m[i] for i in range(3)] + [pc3p[i] for i in range(3)]
        + [pc3n[i] for i in range(3)] + [dsm], axis=1)
    pcs = round_fp32r(pcs)

    bias = conv_b.astype(np.float32).reshape(NCHUNK, 128).T.copy()  # [128, 4]
    return wv, scb, pcs, bias


def build_program(repeat: int = 0, stage: int = 6, conv_bf16: bool = None):
    if conv_bf16 is None:
        conv_bf16 = bool(os.environ.get("KM_BF16"))
    """Build the Bacc program. repeat=0: production. repeat=R>0: wrap the
    compute loop in a hardware For_i executing R times (for timing)."""
    nc = bacc.Bacc()
    CDT = mybir.dt.bfloat16 if conv_bf16 else F32R
    x_in = nc.declare_dram_parameter("x", [E, L], F32, isOutput=False)
    wv_in = nc.declare_dram_parameter("wv", [128, 20 * E], F32, isOutput=False)
    scb_in = nc.declare_dram_parameter("scb", [128, E], F32, isOutput=False)
    pcs_in = nc.declare_dram_parameter("pcs", [128, 1472], F32, isOutput=False)
    bias_in = nc.declare_dram_parameter("bias", [128, NCHUNK], F32, isOutput=False)
    o_out = nc.declare_dram_parameter("out", [E, MOUT], F32, isOutput=True)

    with TileContext(nc) as tc:
        with tc.tile_pool(name="const", bufs=1) as cpool, \
             tc.tile_pool(name="xp", bufs=1) as xpool, \
             tc.tile_pool(name="yp", bufs=int(os.environ.get("KM_YP", "6"))) as ypool, \
             tc.tile_pool(name="scp", bufs=int(os.environ.get("KM_SCP", "4"))) as scpool, \
             tc.tile_pool(name="work", bufs=int(os.environ.get("KM_WK", "3"))) as work, \
             tc.tile_pool(name="outp", bufs=1) as outp, \
             tc.tile_pool(name="psy", bufs=2, space="PSUM") as psy, \
             tc.tile_pool(name="ps2", bufs=int(os.environ.get("KM_PS2", "1")), space="PSUM") as ps2, \
             tc.tile_pool(name="ps3", bufs=int(os.environ.get("KM_PS3", "1")), space="PSUM") as ps3, \
             tc.tile_pool(name="ps4", bufs=1, space="PSUM") as ps4, \
             tc.tile_pool(name="psm", bufs=int(os.environ.get("KM_PSM", "2")), space="PSUM") as psm:

            wv_sb = cpool.tile([128, 20 * E], CDT)
            scb_sb = cpool.tile([128, E], F32)
            pcs_sb = cpool.tile([128, 1472], F32R)
            bias_sb = cpool.tile([128, NCHUNK], F32)
            if conv_bf16:
                wv_f = cpool.tile([128, 20 * E], F32, name="wv_f")
                nc.sync.dma_start(out=wv_f[:], in_=wv_in[:])
                nc.vector.tensor_copy(wv_sb[:], wv_f[:])
            else:
                nc.sync.dma_start(out=wv_sb[:], in_=wv_in[:].bitcast(F32R))
            nc.sync.dma_start(out=scb_sb[:], in_=scb_in[:])
            nc.sync.dma_start(out=pcs_sb[:], in_=pcs_in[:].bitcast(F32R))
            nc.sync.dma_start(out=bias_sb[:], in_=bias_in[:])
            PC2 = pcs_sb[:, 0:128]
            PC4 = pcs_sb[:, 128:256]
            PC3M = [pcs_sb[:, 256 + 128 * i:384 + 128 * i] for i in range(3)]
            PC3P = [pcs_sb[:, 640 + 128 * i:768 + 128 * i] for i in range(3)]
            PC3N = [pcs_sb[:, 1024 + 128 * i:1152 + 128 * i] for i in range(3)]
            DSM = pcs_sb[:, 1408:1472]

            # x quarters with 4-col halo: q0..2: [128, 1028], q3: [128, 1024]
            xq = []
            for c in range(NCHUNK):
                row = []
                for q in range(4):
                    wq = 1028 if q < 3 else 1024
                    xt = xpool.tile([128, wq], CDT, name=f"xq_{c}_{q}")
                    if conv_bf16:
                        xtf = xpool.tile([128, wq], F32, name=f"xqf_{c}_{q}")
                        nc.sync.dma_start(
                            out=xtf[:],
                            in_=x_in[128 * c:128 * (c + 1),
                                     1024 * q:1024 * q + wq])
                        nc.vector.tensor_copy(xt[:], xtf[:])
                    else:
                        nc.sync.dma_start(
                            out=xt[:],
                            in_=x_in[128 * c:128 * (c + 1),
                                     1024 * q:1024 * q + wq].bitcast(F32R))
                    row.append(xt)
                xq.append(row)

            out_sb = []
            for c in range(NCHUNK):
                ot = outp.tile([128, MOUT], F32, name=f"out_sb_{c}")
                if stage < 6:
                    nc.vector.memset(ot[:], 0.0)
                else:
                    nc.vector.memset(ot[:, MOUT - 2:MOUT], 0.0)
                out_sb.append(ot)

            yts = [None] * NT      # yT tiles (SBUF, f32r)
            scs = [None] * NT      # sc1 tiles [128, 2] (SBUF, f32r; col 0 valid)

            def rows_of(t):
                return 124 if t == NT - 1 else 128

            def conv_tile(t):
                r = rows_of(t)
                q, lt = t // 8, t % 8
                py = psy.tile([128, E], F32, name="py")
                for j in range(20):
                    c, k = j // K, j % K
                    lhsT = xq[c][q][:, 128 * lt + k:128 * lt + k + r]
                    nc.tensor.matmul(py[0:r, :], lhsT,
                                     wv_sb[:, E * j:E * (j + 1)],
                                     start=(j == 0), stop=(j == 19))
                yt = ypool.tile([128, E], F32R, name="yt")
                nc.scalar.copy(yt[0:r, :], py[0:r, :])
                if stage < 1:
                    yts[t] = yt
                    scs[t] = None
                    return
                # sc1 = yT . score  (per-partition reduce over e)
                sc = scpool.tile([128, 1], F32R, name="sc")
                scr = work.tile([128, E], F32, name="scr")
                with nc.allow_low_precision("accum rounds to f32r (4-byte)"):
                    nc.vector.scalar_tensor_tensor(
                        scr[0:r, :], yt[0:r, :].bitcast(F32), 1.0,
                        scb_sb[0:r, :], op0=ALU.bypass, op1=ALU.mult,
                        accum_out=sc[0:r, 0:1])
                yts[t], scs[t] = yt, sc

            def post_tile(t):
                if stage < 2:
                    return
                r = rows_of(t)
                ph = (128 * t) % 3
                yt = yts[t]
                has_prev = (t % 3 != 0)
                has_next = (t != NT - 1) and (t % 3 != 2)

                p2 = ps2.tile([128, E], F32, name="p2")
                nc.tensor.matmul(p2[0:r, :], PC2[0:r, 0:r], yt[0:r, :],
                                 start=True, stop=True)
                p4 = ps4.tile([128, E], F32, name="p4")
                nc.tensor.matmul(p4[0:r, :], PC4[0:r, 0:r], yt[0:r, :],
                                 start=True, stop=True)
                p3 = ps3.tile([128, E], F32, name="p3")
                nc.tensor.matmul(p3[0:r, :], PC3M[ph][0:r, 0:r], yt[0:r, :],
                                 start=True, stop=not (has_prev or has_next))
                if has_prev:
                    nc.tensor.matmul(p3[0:r, :], PC3P[ph][64:128, 0:r],
                                     yts[t - 1][64:128, :],
                                     start=False, stop=not has_next)
                if has_next:
                    nc.tensor.matmul(p3[0:r, :], PC3N[ph][0:2, 0:r],
                                     yts[t + 1][0:2, :],
                                     start=False, stop=True)

                if stage < 3:
                    return
                # score pooling-expansion into misc psum cols [0:6]
                pm = psm.tile([128, 512], F32, name="pm")
                sct = scs[t]
                nc.tensor.matmul(pm[0:r, 0:2], PC2[0:r, 0:r], sct[0:r, 0:1].broadcast_to((r, 2)),
                                 start=True, stop=True)
                nc.tensor.matmul(pm[0:r, 4:6], PC4[0:r, 0:r], sct[0:r, 0:1].broadcast_to((r, 2)),
                                 start=True, stop=True)
                nc.tensor.matmul(pm[0:r, 2:4], PC3M[ph][0:r, 0:r], sct[0:r, 0:1].broadcast_to((r, 2)),
                                 start=True, stop=not (has_prev or has_next))
                if has_prev:
                    nc.tensor.matmul(pm[0:r, 2:4], PC3P[ph][64:128, 0:r],
                                     scs[t - 1][64:128, 0:1].broadcast_to((64, 2)),
                                     start=False, stop=not has_next)
                if has_next:
                    nc.tensor.matmul(pm[0:r, 2:4], PC3N[ph][0:2, 0:r],
                                     scs[t + 1][0:2, 0:1].broadcast_to((2, 2)),
                                     start=False, stop=True)

                if stage < 4:
                    return
                # softmax over w in column space
                ecols = work.tile([128, 4], F32, name="ecols")
                nc.scalar.activation(ecols[0:r, 0:1], sct[0:r, 0:1].bitcast(F32),
                                     ACTF.Exp)
                pm3 = pm[0:r, 0:6].rearrange("p (a b) -> p a b", b=2)[:, :, 0]
                nc.scalar.activation(ecols[0:r, 1:4], pm3, ACTF.Exp)
                esum = work.tile([128, 1], F32, name="esum")
                nc.vector.tensor_reduce(esum[0:r, :], ecols[0:r, :],
                                        axis=mybir.AxisListType.X, op=ALU.add)
                erec = work.tile([128, 1], F32, name="erec")
                nc.vector.reciprocal(erec[0:r, :], esum[0:r, :])
                acols = work.tile([128, 4], F32, name="acols")
                nc.vector.tensor_scalar_mul(acols[0:r, :], ecols[0:r, :],
                                            erec[0:r, :])

                if stage < 5:
                    return
                # combine: acc = sum_w A_w * P'_w   (f32r out for the ds matmul)
                acc = work.tile([128, E], F32R, name="acc")
                nc.vector.tensor_scalar_mul(acc[0:r, :], yt[0:r, :].bitcast(F32),
                                            acols[0:r, 0:1])
                for w, pw in ((2, p2), (3, p3), (4, p4)):
                    nc.vector.scalar_tensor_tensor(
                        acc[0:r, :], pw[0:r, :], acols[0:r, w - 1:w],
                        acc[0:r, :].bitcast(F32), op0=ALU.mult, op1=ALU.add)

                if stage < 6:
                    return
                # downsample-by-2 + transpose back: psum [128 e, mc] per chunk
                mc = r // 2
                for c in range(NCHUNK):
                    nc.tensor.matmul(pm[:, 64 + 64 * c:64 + 64 * c + mc],
                                     acc[0:r, 128 * c:128 * (c + 1)],
                                     DSM[0:r, 0:mc], start=True, stop=True)
                for c in range(NCHUNK):
                    nc.scalar.activation(
                        out_sb[c][:, 64 * t:64 * t + mc],
                        pm[:, 64 + 64 * c:64 + 64 * c + mc],
                        ACTF.Identity, bias=bias_sb[:, c:c + 1], scale=1.0)

            def body():
                for t in range(NT + 1):
                    if t < NT:
                        conv_tile(t)
                    if t >= 1:
                        post_tile(t - 1)
                    if (t == 16 or t == NT) and not os.environ.get("KM_NODMA"):
                        h = 0 if t == 16 else 1
                        for c in range(NCHUNK):
                            nc.sync.dma_start(
                                out=o_out[128 * c:128 * (c + 1),
                                          1024 * h:1024 * (h + 1)],
                                in_=out_sb[c][:, 1024 * h:1024 * (h + 1)])

            if repeat:
                with tc.For_i(0, repeat, 1) as _i:
                    body()
            else:
                body()

    nc.finalize()
    return nc


_CACHE = {}


def _get_program(repeat=0):
    if repeat not in _CACHE:
        _CACHE[repeat] = build_program(repeat)
    return _CACHE[repeat]


def make_in_maps(x, conv_w, conv_b, score_w):
    wv, scb, pcs, bias = _host_consts(np.asarray(conv_w, np.float32),
                                      np.asarray(conv_b, np.float32),
                                      np.asarray(score_w, np.float32))
    xr = round_fp32r(np.asarray(x, np.float32))
    return [{"x": np.ascontiguousarray(xr[b]), "wv": wv, "scb": scb,
             "pcs": pcs, "bias": bias} for b in range(B)]


def kernel(x, conv_w, conv_b, score_w):
    nc = _get_program()
    in_maps = make_in_maps(x, conv_w, conv_b, score_w)
    res = run_bass_kernel_spmd(nc, in_maps, core_ids=list(range(B)))
    return np.stack([res.results[b]["out"] for b in range(B)], axis=0)



# revision 22
# speedup vs baseline: 2.4695x; 2.4695x over previous
"""GBST (Charformer) layer kernel for Trainium2, 8 NeuronCores, batch-parallel.

Per core (one batch element):
  x (512, 4096) --conv K=5 VALID--> y (512, 4092), computed TRANSPOSED as
  yT tiles [128 l, 512 e] on PSUM (bf16 matmuls, x slices as stationary).
  Scores s1[l] = score . y[:, l] via DVE tensor_tensor_reduce on yT.
  Multi-width pooling (w=2,3,4) + repeat-expansion fused into single PE matmuls
  with constant block-banded matrices (entries 1/w).  Softmax over w runs in
  column space [128 l, 4].  The combine is per-partition DVE ops; conv bias is
  folded into the combine via a broadcast tile (softmax shift-invariance makes
  it drop out of the scores).  Final avg-pool-by-2 + transpose back to [e, m]
  is one PE matmul per e-chunk; all four chunks evict in one activation op.

bf16 is used for all matmul operands and SBUF-resident tensors (PSUM
accumulation stays fp32), giving ~1e-3 end-to-end error.
"""
import os, sys
sys.path.insert(0, "/opt/trn_rl_repo")
import numpy as np
import ml_dtypes

import concourse.bacc as bacc
import concourse.mybir as mybir
from concourse.tile import TileContext
from concourse.bass_utils import run_bass_kernel_spmd

BF16 = mybir.dt.bfloat16
F32 = mybir.dt.float32
NPBF = ml_dtypes.bfloat16
B, E, L = 8, 512, 4096
K = 5
L2 = L - K + 1            # 4092
NT = 32                   # l-tiles of 128 (last has 124 rows)
NCHUNK = 4                # e chunks of 128
MOUT = L // 2             # 2048
ALU = mybir.AluOpType
ACTF = mybir.ActivationFunctionType
_ctx = [""]   # current build phase label, for trace attribution


def _pool3_mats():
    """Per-phase main/prev/next lhsT matrices for w=3 pooling-expansion.

    main[ph][l', l] = 1/3 if global blocks match within the tile (ph = 128t mod 3)
    prev[ph][l', l] : contribution of previous tile's row l' (only l' in {126,127})
    next[ph][l', l] : contribution of next tile's row l' (only l' in {0,1})
    """
    m = np.zeros((3, 128, 128), np.float32)
    p = np.zeros((3, 128, 128), np.float32)
    n = np.zeros((3, 128, 128), np.float32)
    for ph in (0, 1, 2):
        t = [3, 2, 1][ph]          # representative t with (128*t) % 3 == ph
        assert (128 * t) % 3 == ph
        base = 128 * t
        for l in range(128):
            blk = (base + l) // 3
            for lp in range(128):
                if (base + lp) // 3 == blk:
                    m[ph, lp, l] = 1.0 / 3.0
                if (base - 128 + lp) // 3 == blk:
                    p[ph, lp, l] = 1.0 / 3.0
                if (base + 128 + lp) // 3 == blk:
                    n[ph, lp, l] = 1.0 / 3.0
    assert np.all(p[:, :126, :] == 0) and np.all(n[:, 2:, :] == 0)
    return m, p, n


def _host_consts(conv_w, conv_b, score_w):
    # wv: [128, 20*512]; block j = 5*c + k holds wT[k, chunk c]: [i, e] = w[e, 128c+i, k]
    wt = conv_w.transpose(1, 0, 2).reshape(NCHUNK, 128, E, K)   # (c, i, e, k)
    wv = wt.transpose(1, 0, 3, 2).reshape(128, NCHUNK * K * E)  # i, (c,k), e
    wv = wv.astype(NPBF)

    scb = np.broadcast_to(score_w.astype(NPBF), (128, E)).copy()

    pc2 = np.zeros((128, 128), np.float32)
    pc4 = np.zeros((128, 128), np.float32)
    for l in range(128):
        pc2[(l // 2) * 2:(l // 2) * 2 + 2, l] = 0.5
        pc4[(l // 4) * 4:(l // 4) * 4 + 4, l] = 0.25
    pc3m, pc3p, pc3n = _pool3_mats()
    dsm = np.zeros((128, 64), np.float32)
    for l in range(128):
        dsm[l, l // 2] = 0.5
    # pack: pc2 [0:128], pc4 [128:256], pc3m [256:640], pc3p [640:1024],
    #       pc3n [1024:1408], dsm [1408:1472]
    pcs = np.concatenate(
        [pc2, pc4] + [pc3m[i] for i in range(3)] + [pc3p[i] for i in range(3)]
        + [pc3n[i] for i in range(3)] + [dsm], axis=1).astype(NPBF)

    bias_bc = np.broadcast_to(conv_b.astype(NPBF), (128, E)).copy()
    return wv, scb, pcs, bias_bc


def build_program(repeat: int = 0, stage: int = 6):
    """Build the Bacc program. repeat=0: production. repeat=R>0: wrap the
    compute loop in a hardware For_i executing R times (for timing)."""
    nc = bacc.Bacc()
    x_in = nc.declare_dram_parameter("x", [E, L], BF16, isOutput=False)
    wv_in = nc.declare_dram_parameter("wv", [128, 20 * E], BF16, isOutput=False)
    scb_in = nc.declare_dram_parameter("scb", [128, E], BF16, isOutput=False)
    pcs_in = nc.declare_dram_parameter("pcs", [128, 1472], BF16, isOutput=False)
    bias_in = nc.declare_dram_parameter("bias_bc", [128, E], BF16, isOutput=False)
    o_out = nc.declare_dram_parameter("out", [E, MOUT], F32, isOutput=True)

    with TileContext(nc) as tc:
        with tc.tile_pool(name="const", bufs=1) as cpool, \
             tc.tile_pool(name="xp", bufs=1) as xpool, \
             tc.tile_pool(name="yp", bufs=int(os.environ.get("KM_YP", "6"))) as ypool, \
             tc.tile_pool(name="scp", bufs=int(os.environ.get("KM_SCP", "4"))) as scpool, \
             tc.tile_pool(name="work", bufs=int(os.environ.get("KM_WK", "3"))) as work, \
             tc.tile_pool(name="outp", bufs=1) as outp, \
             tc.tile_pool(name="psy", bufs=2, space="PSUM") as psy, \
             tc.tile_pool(name="ps2", bufs=int(os.environ.get("KM_PS2", "1")), space="PSUM") as ps2, \
             tc.tile_pool(name="ps3", bufs=int(os.environ.get("KM_PS3", "1")), space="PSUM") as ps3, \
             tc.tile_pool(name="ps4", bufs=int(os.environ.get("KM_PS4", "2")), space="PSUM") as ps4, \
             tc.tile_pool(name="psm", bufs=int(os.environ.get("KM_PSM", "1")), space="PSUM") as psm, \
             tc.tile_pool(name="psd", bufs=int(os.environ.get("KM_PSD", "1")), space="PSUM") as psd:

            wv_sb = cpool.tile([128, 20 * E], BF16)
            scb_sb = cpool.tile([128, E], BF16)
            pcs_sb = cpool.tile([128, 1472], BF16)
            bias_sb = cpool.tile([128, E], BF16)
            # first conv tile needs wv block 0 + the q=0 x quarters ASAP;
            # split the wv transfer so compute starts after ~2 small DMAs
            nc.sync.dma_start(out=wv_sb[:, 0:5 * E], in_=wv_in[:, 0:5 * E])
            PC2 = pcs_sb[:, 0:128]
            PC4 = pcs_sb[:, 128:256]
            PC3M = [pcs_sb[:, 256 + 128 * i:384 + 128 * i] for i in range(3)]
            PC3P = [pcs_sb[:, 640 + 128 * i:768 + 128 * i] for i in range(3)]
            PC3N = [pcs_sb[:, 1024 + 128 * i:1152 + 128 * i] for i in range(3)]
            DSM = pcs_sb[:, 1408:1472]

            # x quarters with 4-col halo: q0..2: [128, 1028], q3: [128, 1024]
            # DMA in q-major order so tile 0's operands (q=0, all c) land first
            xq = [[None] * 4 for _ in range(NCHUNK)]
            for q in range(4):
                for c in range(NCHUNK):
                    wq = 1028 if q < 3 else 1024
                    xt = xpool.tile([128, wq], BF16, name=f"xq_{c}_{q}")
                    nc.sync.dma_start(
                        out=xt[:],
                        in_=x_in[128 * c:128 * (c + 1), 1024 * q:1024 * q + wq])
                    xq[c][q] = xt
                if q == 0:
                    for j in range(1, 4):
                        nc.sync.dma_start(out=wv_sb[:, 5 * E * j:5 * E * (j + 1)],
                                          in_=wv_in[:, 5 * E * j:5 * E * (j + 1)])
                    nc.sync.dma_start(out=scb_sb[:], in_=scb_in[:])
                    nc.sync.dma_start(out=pcs_sb[:], in_=pcs_in[:])
                    nc.sync.dma_start(out=bias_sb[:], in_=bias_in[:])

            out_sb = outp.tile([128, NCHUNK * MOUT], F32, name="out_sb")
            ot3 = out_sb[:].rearrange("p (c m) -> p c m", m=MOUT)
            if stage < 6:
                nc.vector.memset(out_sb[:], 0.0)
            else:
                nc.vector.memset(ot3[:, :, MOUT - 2:MOUT], 0.0)

            yts = [None] * NT      # yT tiles (SBUF, bf16)
            scs = [None] * NT      # sc1 tiles [128, 1] (SBUF, bf16)
            accs = [None] * NT     # combined tiles (SBUF, bf16)

            def rows_of(t):
                return 124 if t == NT - 1 else 128

            def conv_tile(t):
                _ctx[0] = f"conv.{t}"
                r = rows_of(t)
                q, lt = t // 8, t % 8
                py = psy.tile([128, E], F32, name="py")
                for j in range(20):
                    c, k = j // K, j % K
                    lhsT = xq[c][q][:, 128 * lt + k:128 * lt + k + r]
                    nc.tensor.matmul(py[0:r, :], lhsT,
                                     wv_sb[:, E * j:E * (j + 1)],
                                     start=(j == 0), stop=(j == 19))
                yt = ypool.tile([128, E], BF16, name="yt")
                nc.scalar.copy(yt[0:r, :], py[0:r, :])
                if stage < 1:
                    yts[t] = yt
                    scs[t] = None
                    return
                # sc1 = yT . score  (per-partition reduce over e)
                sc = scpool.tile([128, 1], BF16, name="sc")
                scr = work.tile([128, E], BF16, name="scr")
                with nc.allow_low_precision("accum rounds to bf16 (2-byte)"):
                    nc.vector.scalar_tensor_tensor(
                        scr[0:r, :], yt[0:r, :], 1.0,
                        scb_sb[0:r, :], op0=ALU.bypass, op1=ALU.mult,
                        accum_out=sc[0:r, 0:1])
                yts[t], scs[t] = yt, sc

            p2s = [None] * NT
            p3s = [None] * NT
            p4s = [None] * NT
            pms = [None] * NT

            def has_prev(t):
                return t % 3 != 0

            def has_next(t):
                return (t != NT - 1) and (t % 3 != 2)

            def post_a(t):
                """Pooling + score-pooling matmuls except the next-halo ones."""
                _ctx[0] = f"postA.{t}"
                if stage < 2:
                    return
                r = rows_of(t)
                ph = (128 * t) % 3
                yt = yts[t]

                p2 = ps2.tile([128, E], F32, name="p2")
                nc.tensor.matmul(p2[0:r, :], PC2[0:r, 0:r], yt[0:r, :],
                                 start=True, stop=True)
                p4 = ps4.tile([128, E], F32, name="p4")
                nc.tensor.matmul(p4[0:r, :], PC4[0:r, 0:r], yt[0:r, :],
                                 start=True, stop=True)
                p3 = ps3.tile([128, E], F32, name="p3")
                nc.tensor.matmul(p3[0:r, :], PC3M[ph][0:r, 0:r], yt[0:r, :],
                                 start=True,
                                 stop=not (has_prev(t) or has_next(t)))
                if has_prev(t):
                    nc.tensor.matmul(p3[0:r, :], PC3P[ph][64:128, 0:r],
                                     yts[t - 1][64:128, :],
                                     start=False, stop=not has_next(t))
                p2s[t], p3s[t], p4s[t] = p2, p3, p4

                if stage < 3:
                    return
                # score pooling-expansion into misc psum cols [0:6]
                pm = psm.tile([128, 8], F32, name="pm")
                sct = scs[t]
                nc.tensor.matmul(pm[0:r, 0:2], PC2[0:r, 0:r], sct[0:r, 0:1].broadcast_to((r, 2)),
                                 start=True, stop=True)
                nc.tensor.matmul(pm[0:r, 4:6], PC4[0:r, 0:r], sct[0:r, 0:1].broadcast_to((r, 2)),
                                 start=True, stop=True)
                nc.tensor.matmul(pm[0:r, 2:4], PC3M[ph][0:r, 0:r], sct[0:r, 0:1].broadcast_to((r, 2)),
                                 start=True,
                                 stop=not (has_prev(t) or has_next(t)))
                if has_prev(t):
                    nc.tensor.matmul(pm[0:r, 2:4], PC3P[ph][64:128, 0:r],
                                     scs[t - 1][64:128, 0:1].broadcast_to((64, 2)),
                                     start=False, stop=not has_next(t))
                pms[t] = pm

            def post_b(t):
                """Next-halo matmuls (deps settled a full tile ago) + softmax
                + combine.  Issued before the conv streams of step t+2 so the
                PE never waits on the freshly evicted y tile."""
                _ctx[0] = f"postB.{t}"
                if stage < 2:
                    return
                r = rows_of(t)
                ph = (128 * t) % 3
                yt = yts[t]
                p2, p3, p4, pm = p2s[t], p3s[t], p4s[t], pms[t]
                if has_next(t):
                    nc.tensor.matmul(p3[0:r, :], PC3N[ph][0:2, 0:r],
                                     yts[t + 1][0:2, :],
                                     start=False, stop=True)
                    if stage >= 3:
                        nc.tensor.matmul(pm[0:r, 2:4], PC3N[ph][0:2, 0:r],
                                         scs[t + 1][0:2, 0:1].broadcast_to((2, 2)),
                                         start=False, stop=True)

                if stage < 4:
                    return
                # softmax over w in column space
                sct = scs[t]
                ecols = work.tile([128, 4], F32, name="ecols")
                nc.scalar.activation(ecols[0:r, 0:1], sct[0:r, 0:1], ACTF.Exp)
                pm3 = pm[0:r, 0:6].rearrange("p (a b) -> p a b", b=2)[:, :, 0]
                nc.scalar.activation(ecols[0:r, 1:4], pm3, ACTF.Exp)
                esum = work.tile([128, 1], F32, name="esum")
                nc.vector.tensor_reduce(esum[0:r, :], ecols[0:r, :],
                                        axis=mybir.AxisListType.X, op=ALU.add)
                erec = work.tile([128, 1], F32, name="erec")
                nc.vector.reciprocal(erec[0:r, :], esum[0:r, :])
                acols = work.tile([128, 4], F32, name="acols")
                nc.vector.tensor_scalar_mul(acols[0:r, :], ecols[0:r, :],
                                            erec[0:r, :])

                if stage < 5:
                    return
                # combine: acc = sum_w A_w * P'_w + bias  (bf16 out for ds matmul)
                # op1 (all-SBUF) runs on the otherwise-idle GPSIMD engine
                acc = work.tile([128, E], BF16, name="acc")
                if os.environ.get("KM_POOL"):
                    nc.gpsimd.scalar_tensor_tensor(
                        acc[0:r, :], yt[0:r, :], acols[0:r, 0:1],
                        bias_sb[0:r, :], op0=ALU.mult, op1=ALU.add)
                else:
                    nc.vector.scalar_tensor_tensor(
                        acc[0:r, :], yt[0:r, :], acols[0:r, 0:1],
                        bias_sb[0:r, :], op0=ALU.mult, op1=ALU.add)
                for w, pw in ((2, p2), (3, p3), (4, p4)):
                    nc.vector.scalar_tensor_tensor(
                        acc[0:r, :], pw[0:r, :], acols[0:r, w - 1:w],
                        acc[0:r, :], op0=ALU.mult, op1=ALU.add)
                accs[t] = acc

            def ds_tile(t):
                _ctx[0] = f"ds.{t}"
                # downsample-by-2 + transpose back: psum [128 e, mc] per chunk.
                # Runs two steps behind conv so the PE queue never head-blocks
                # on the DVE combine chain.
                if stage < 6:
                    return
                r = rows_of(t)
                acc = accs[t]
                mc = r // 2
                pd = psd.tile([128, 256], F32, name="pd")
                for c in range(NCHUNK):
                    nc.tensor.matmul(pd[:, mc * c:mc * (c + 1)],
                                     acc[0:r, 128 * c:128 * (c + 1)],
                                     DSM[0:r, 0:mc], start=True, stop=True)
                nc.scalar.copy(
                    ot3[:, :, 64 * t:64 * t + mc],
                    pd[:, 0:4 * mc].rearrange("p (c m) -> p c m", m=mc))

            def body():
                for t in range(NT + 3):
                    if 2 <= t <= NT + 1:
                        post_b(t - 2)
                    if t < NT:
                        conv_tile(t)
                    if 1 <= t <= NT:
                        post_a(t - 1)
                    if t >= 3:
                        ds_tile(t - 3)
                    # flush output columns as their tiles finish downsampling
                    flush = {18: (0, 1024), 27: (1024, 512),
                             31: (1536, 256), NT + 2: (1792, 256)}
                    if t in flush and not os.environ.get("KM_NODMA"):
                        m0, mw = flush[t]
                        for c in range(NCHUNK):
                            nc.sync.dma_start(
                                out=o_out[128 * c:128 * (c + 1), m0:m0 + mw],
                                in_=out_sb[:, MOUT * c + m0:MOUT * c + m0 + mw])

            if repeat:
                with tc.For_i(0, repeat, 1) as _i:
                    body()
            else:
                body()

    nc.finalize()
    return nc


_CACHE = {}


def _get_program(repeat=0):
    if repeat not in _CACHE:
        _CACHE[repeat] = build_program(repeat)
    return _CACHE[repeat]


def make_in_maps(x, conv_w, conv_b, score_w):
    wv, scb, pcs, bias_bc = _host_consts(np.asarray(conv_w, np.float32),
                                         np.asarray(conv_b, np.float32),
                                         np.asarray(score_w, np.float32))
    xb = np.asarray(x, np.float32).astype(NPBF)
    return [{"x": np.ascontiguousarray(xb[b]), "wv": wv, "scb": scb,
             "pcs": pcs, "bias_bc": bias_bc} for b in range(B)]


def kernel(x, conv_w, conv_b, score_w):
    nc = _get_program()
    in_maps = make_in_maps(x, conv_w, conv_b, score_w)
    res = run_bass_kernel_spmd(nc, in_maps, core_ids=list(range(B)))
    return np.stack([res.results[b]["out"] for b in range(B)], axis=0)


# revision 23
# speedup vs baseline: 2.4758x; 1.0026x over previous
"""GBST (Charformer) layer kernel for Trainium2, 8 NeuronCores, batch-parallel.

Per core (one batch element):
  x (512, 4096) --conv K=5 VALID--> y (512, 4092), computed TRANSPOSED as
  yT tiles [128 l, 512 e] on PSUM (bf16 matmuls, x slices as stationary).
  Scores s1[l] = score . y[:, l] via DVE tensor_tensor_reduce on yT.
  Multi-width pooling (w=2,3,4) + repeat-expansion fused into single PE matmuls
  with constant block-banded matrices (entries 1/w).  Softmax over w runs in
  column space [128 l, 4].  The combine is per-partition DVE ops; conv bias is
  folded into the combine via a broadcast tile (softmax shift-invariance makes
  it drop out of the scores).  Final avg-pool-by-2 + transpose back to [e, m]
  is one PE matmul per e-chunk; all four chunks evict in one activation op.

bf16 is used for all matmul operands and SBUF-resident tensors (PSUM
accumulation stays fp32), giving ~1e-3 end-to-end error.
"""
import os, sys
sys.path.insert(0, "/opt/trn_rl_repo")
import numpy as np
import ml_dtypes

import concourse.bacc as bacc
import concourse.mybir as mybir
from concourse.tile import TileContext
from concourse.bass_utils import run_bass_kernel_spmd

BF16 = mybir.dt.bfloat16
F32 = mybir.dt.float32
NPBF = ml_dtypes.bfloat16
B, E, L = 8, 512, 4096
K = 5
L2 = L - K + 1            # 4092
NT = 32                   # l-tiles of 128 (last has 124 rows)
NCHUNK = 4                # e chunks of 128
MOUT = L // 2             # 2048
ALU = mybir.AluOpType
ACTF = mybir.ActivationFunctionType
_ctx = [""]   # current build phase label, for trace attribution


def _pool3_mats():
    """Per-phase main/prev/next lhsT matrices for w=3 pooling-expansion.

    main[ph][l', l] = 1/3 if global blocks match within the tile (ph = 128t mod 3)
    prev[ph][l', l] : contribution of previous tile's row l' (only l' in {126,127})
    next[ph][l', l] : contribution of next tile's row l' (only l' in {0,1})
    """
    m = np.zeros((3, 128, 128), np.float32)
    p = np.zeros((3, 128, 128), np.float32)
    n = np.zeros((3, 128, 128), np.float32)
    for ph in (0, 1, 2):
        t = [3, 2, 1][ph]          # representative t with (128*t) % 3 == ph
        assert (128 * t) % 3 == ph
        base = 128 * t
        for l in range(128):
            blk = (base + l) // 3
            for lp in range(128):
                if (base + lp) // 3 == blk:
                    m[ph, lp, l] = 1.0 / 3.0
                if (base - 128 + lp) // 3 == blk:
                    p[ph, lp, l] = 1.0 / 3.0
                if (base + 128 + lp) // 3 == blk:
                    n[ph, lp, l] = 1.0 / 3.0
    assert np.all(p[:, :126, :] == 0) and np.all(n[:, 2:, :] == 0)
    return m, p, n


def _host_consts(conv_w, conv_b, score_w):
    # wv: [128, 20*512]; block j = 5*c + k holds wT[k, chunk c]: [i, e] = w[e, 128c+i, k]
    wt = conv_w.transpose(1, 0, 2).reshape(NCHUNK, 128, E, K)   # (c, i, e, k)
    wv = wt.transpose(1, 0, 3, 2).reshape(128, NCHUNK * K * E)  # i, (c,k), e
    wv = wv.astype(NPBF)

    scb = np.broadcast_to(score_w.astype(NPBF), (128, E)).copy()

    pc2 = np.zeros((128, 128), np.float32)
    pc4 = np.zeros((128, 128), np.float32)
    for l in range(128):
        pc2[(l // 2) * 2:(l // 2) * 2 + 2, l] = 0.5
        pc4[(l // 4) * 4:(l // 4) * 4 + 4, l] = 0.25
    pc3m, pc3p, pc3n = _pool3_mats()
    dsm = np.zeros((128, 64), np.float32)
    for l in range(128):
        dsm[l, l // 2] = 0.5
    # pack: pc2 [0:128], pc4 [128:256], pc3m [256:640], pc3p [640:1024],
    #       pc3n [1024:1408], dsm [1408:1472]
    pcs = np.concatenate(
        [pc2, pc4] + [pc3m[i] for i in range(3)] + [pc3p[i] for i in range(3)]
        + [pc3n[i] for i in range(3)] + [dsm], axis=1).astype(NPBF)

    bias_bc = np.broadcast_to(conv_b.astype(NPBF), (128, E)).copy()
    return wv, scb, pcs, bias_bc


def build_program(repeat: int = 0, stage: int = 6):
    """Build the Bacc program. repeat=0: production. repeat=R>0: wrap the
    compute loop in a hardware For_i executing R times (for timing)."""
    nc = bacc.Bacc()
    x_in = nc.declare_dram_parameter("x", [E, L], BF16, isOutput=False)
    wv_in = nc.declare_dram_parameter("wv", [128, 20 * E], BF16, isOutput=False)
    scb_in = nc.declare_dram_parameter("scb", [128, E], BF16, isOutput=False)
    pcs_in = nc.declare_dram_parameter("pcs", [128, 1472], BF16, isOutput=False)
    bias_in = nc.declare_dram_parameter("bias_bc", [128, E], BF16, isOutput=False)
    o_out = nc.declare_dram_parameter("out", [E, MOUT], F32, isOutput=True)

    with TileContext(nc) as tc:
        with tc.tile_pool(name="const", bufs=1) as cpool, \
             tc.tile_pool(name="xp", bufs=1) as xpool, \
             tc.tile_pool(name="yp", bufs=int(os.environ.get("KM_YP", "6"))) as ypool, \
             tc.tile_pool(name="scp", bufs=int(os.environ.get("KM_SCP", "4"))) as scpool, \
             tc.tile_pool(name="work", bufs=int(os.environ.get("KM_WK", "3"))) as work, \
             tc.tile_pool(name="outp", bufs=1) as outp, \
             tc.tile_pool(name="psy", bufs=2, space="PSUM") as psy, \
             tc.tile_pool(name="ps2", bufs=int(os.environ.get("KM_PS2", "1")), space="PSUM") as ps2, \
             tc.tile_pool(name="ps3", bufs=int(os.environ.get("KM_PS3", "1")), space="PSUM") as ps3, \
             tc.tile_pool(name="ps4", bufs=int(os.environ.get("KM_PS4", "2")), space="PSUM") as ps4, \
             tc.tile_pool(name="psm", bufs=int(os.environ.get("KM_PSM", "1")), space="PSUM") as psm, \
             tc.tile_pool(name="psd", bufs=int(os.environ.get("KM_PSD", "1")), space="PSUM") as psd:

            wv_sb = cpool.tile([128, 20 * E], BF16)
            scb_sb = cpool.tile([128, E], BF16)
            pcs_sb = cpool.tile([128, 1472], BF16)
            bias_sb = cpool.tile([128, E], BF16)
            # first conv tile needs wv block 0 + the q=0 x quarters ASAP;
            # split the wv transfer so compute starts after ~2 small DMAs
            nc.sync.dma_start(out=wv_sb[:, 0:5 * E], in_=wv_in[:, 0:5 * E])
            PC2 = pcs_sb[:, 0:128]
            PC4 = pcs_sb[:, 128:256]
            PC3M = [pcs_sb[:, 256 + 128 * i:384 + 128 * i] for i in range(3)]
            PC3P = [pcs_sb[:, 640 + 128 * i:768 + 128 * i] for i in range(3)]
            PC3N = [pcs_sb[:, 1024 + 128 * i:1152 + 128 * i] for i in range(3)]
            DSM = pcs_sb[:, 1408:1472]

            # x quarters with 4-col halo: q0..2: [128, 1028], q3: [128, 1024]
            # DMA in q-major order so tile 0's operands (q=0, all c) land first
            xq = [[None] * 4 for _ in range(NCHUNK)]
            for q in range(4):
                for c in range(NCHUNK):
                    wq = 1028 if q < 3 else 1024
                    xt = xpool.tile([128, wq], BF16, name=f"xq_{c}_{q}")
                    nc.sync.dma_start(
                        out=xt[:],
                        in_=x_in[128 * c:128 * (c + 1), 1024 * q:1024 * q + wq])
                    xq[c][q] = xt
                if q == 0:
                    for j in range(1, 4):
                        nc.sync.dma_start(out=wv_sb[:, 5 * E * j:5 * E * (j + 1)],
                                          in_=wv_in[:, 5 * E * j:5 * E * (j + 1)])
                    nc.sync.dma_start(out=scb_sb[:], in_=scb_in[:])
                    nc.sync.dma_start(out=pcs_sb[:], in_=pcs_in[:])
                    nc.sync.dma_start(out=bias_sb[:], in_=bias_in[:])

            out_sb = outp.tile([128, NCHUNK * MOUT], F32, name="out_sb")
            ot3 = out_sb[:].rearrange("p (c m) -> p c m", m=MOUT)
            if stage < 6:
                nc.vector.memset(out_sb[:], 0.0)
            else:
                nc.vector.memset(ot3[:, :, MOUT - 2:MOUT], 0.0)

            yts = [None] * NT      # yT tiles (SBUF, bf16)
            scs = [None] * NT      # sc1 tiles [128, 1] (SBUF, bf16)
            accs = [None] * NT     # combined tiles (SBUF, bf16)

            def rows_of(t):
                return 124 if t == NT - 1 else 128

            def conv_tile(t):
                _ctx[0] = f"conv.{t}"
                r = rows_of(t)
                q, lt = t // 8, t % 8
                py = psy.tile([128, E], F32, name="py")
                for j in range(20):
                    c, k = j // K, j % K
                    lhsT = xq[c][q][:, 128 * lt + k:128 * lt + k + r]
                    nc.tensor.matmul(py[0:r, :], lhsT,
                                     wv_sb[:, E * j:E * (j + 1)],
                                     start=(j == 0), stop=(j == 19))
                yt = ypool.tile([128, E], BF16, name="yt")
                nc.scalar.copy(yt[0:r, :], py[0:r, :])
                if stage < 1:
                    yts[t] = yt
                    scs[t] = None
                    return
                # sc1 = yT . score  (per-partition reduce over e)
                sc = scpool.tile([128, 1], BF16, name="sc")
                scr = work.tile([128, E], BF16, name="scr")
                with nc.allow_low_precision("accum rounds to bf16 (2-byte)"):
                    nc.vector.scalar_tensor_tensor(
                        scr[0:r, :], yt[0:r, :], 1.0,
                        scb_sb[0:r, :], op0=ALU.bypass, op1=ALU.mult,
                        accum_out=sc[0:r, 0:1])
                yts[t], scs[t] = yt, sc

            p2s = [None] * NT
            p3s = [None] * NT
            p4s = [None] * NT
            pms = [None] * NT

            def has_prev(t):
                return t % 3 != 0

            def has_next(t):
                return (t != NT - 1) and (t % 3 != 2)

            def post_a(t):
                """Pooling + score-pooling matmuls except the next-halo ones."""
                _ctx[0] = f"postA.{t}"
                if stage < 2:
                    return
                r = rows_of(t)
                ph = (128 * t) % 3
                yt = yts[t]

                p2 = ps2.tile([128, E], F32, name="p2")
                nc.tensor.matmul(p2[0:r, :], PC2[0:r, 0:r], yt[0:r, :],
                                 start=True, stop=True)
                p4 = ps4.tile([128, E], F32, name="p4")
                nc.tensor.matmul(p4[0:r, :], PC4[0:r, 0:r], yt[0:r, :],
                                 start=True, stop=True)
                p3 = ps3.tile([128, E], F32, name="p3")
                nc.tensor.matmul(p3[0:r, :], PC3M[ph][0:r, 0:r], yt[0:r, :],
                                 start=True,
                                 stop=not (has_prev(t) or has_next(t)))
                if has_prev(t):
                    nc.tensor.matmul(p3[0:r, :], PC3P[ph][64:128, 0:r],
                                     yts[t - 1][64:128, :],
                                     start=False, stop=not has_next(t))
                p2s[t], p3s[t], p4s[t] = p2, p3, p4

                if stage < 3:
                    return
                # score pooling-expansion into misc psum cols [0:6]
                pm = psm.tile([128, 8], F32, name="pm")
                sct = scs[t]
                nc.tensor.matmul(pm[0:r, 0:2], PC2[0:r, 0:r], sct[0:r, 0:1].broadcast_to((r, 2)),
                                 start=True, stop=True)
                nc.tensor.matmul(pm[0:r, 4:6], PC4[0:r, 0:r], sct[0:r, 0:1].broadcast_to((r, 2)),
                                 start=True, stop=True)
                nc.tensor.matmul(pm[0:r, 2:4], PC3M[ph][0:r, 0:r], sct[0:r, 0:1].broadcast_to((r, 2)),
                                 start=True,
                                 stop=not (has_prev(t) or has_next(t)))
                if has_prev(t):
                    nc.tensor.matmul(pm[0:r, 2:4], PC3P[ph][64:128, 0:r],
                                     scs[t - 1][64:128, 0:1].broadcast_to((64, 2)),
                                     start=False, stop=not has_next(t))
                pms[t] = pm

            def post_b(t):
                """Next-halo matmuls (deps settled a full tile ago) + softmax
                + combine.  Issued before the conv streams of step t+2 so the
                PE never waits on the freshly evicted y tile."""
                _ctx[0] = f"postB.{t}"
                if stage < 2:
                    return
                r = rows_of(t)
                ph = (128 * t) % 3
                yt = yts[t]
                p2, p3, p4, pm = p2s[t], p3s[t], p4s[t], pms[t]
                if has_next(t):
                    nc.tensor.matmul(p3[0:r, :], PC3N[ph][0:2, 0:r],
                                     yts[t + 1][0:2, :],
                                     start=False, stop=True)
                    if stage >= 3:
                        nc.tensor.matmul(pm[0:r, 2:4], PC3N[ph][0:2, 0:r],
                                         scs[t + 1][0:2, 0:1].broadcast_to((2, 2)),
                                         start=False, stop=True)

                if stage < 4:
                    return
                # softmax over w in column space
                sct = scs[t]
                ecols = work.tile([128, 4], F32, name="ecols")
                nc.scalar.activation(ecols[0:r, 0:1], sct[0:r, 0:1], ACTF.Exp)
                pm3 = pm[0:r, 0:6].rearrange("p (a b) -> p a b", b=2)[:, :, 0]
                nc.scalar.activation(ecols[0:r, 1:4], pm3, ACTF.Exp)
                esum = work.tile([128, 1], F32, name="esum")
                nc.vector.tensor_reduce(esum[0:r, :], ecols[0:r, :],
                                        axis=mybir.AxisListType.X, op=ALU.add)
                erec = work.tile([128, 1], F32, name="erec")
                nc.vector.reciprocal(erec[0:r, :], esum[0:r, :])
                acols = work.tile([128, 4], F32, name="acols")
                nc.scalar.activation(acols[0:r, :], ecols[0:r, :],
                                     ACTF.Identity, scale=erec[0:r, 0:1])

                if stage < 5:
                    return
                # combine: acc = sum_w A_w * P'_w + bias  (bf16 out for ds matmul)
                # op1 (all-SBUF) runs on the otherwise-idle GPSIMD engine
                acc = work.tile([128, E], BF16, name="acc")
                if os.environ.get("KM_POOL"):
                    nc.gpsimd.scalar_tensor_tensor(
                        acc[0:r, :], yt[0:r, :], acols[0:r, 0:1],
                        bias_sb[0:r, :], op0=ALU.mult, op1=ALU.add)
                else:
                    nc.vector.scalar_tensor_tensor(
                        acc[0:r, :], yt[0:r, :], acols[0:r, 0:1],
                        bias_sb[0:r, :], op0=ALU.mult, op1=ALU.add)
                for w, pw in ((2, p2), (3, p3), (4, p4)):
                    nc.vector.scalar_tensor_tensor(
                        acc[0:r, :], pw[0:r, :], acols[0:r, w - 1:w],
                        acc[0:r, :], op0=ALU.mult, op1=ALU.add)
                accs[t] = acc

            def ds_tile(t):
                _ctx[0] = f"ds.{t}"
                # downsample-by-2 + transpose back: psum [128 e, mc] per chunk.
                # Runs two steps behind conv so the PE queue never head-blocks
                # on the DVE combine chain.
                if stage < 6:
                    return
                r = rows_of(t)
                acc = accs[t]
                mc = r // 2
                pd = psd.tile([128, 256], F32, name="pd")
                for c in range(NCHUNK):
                    nc.tensor.matmul(pd[:, mc * c:mc * (c + 1)],
                                     acc[0:r, 128 * c:128 * (c + 1)],
                                     DSM[0:r, 0:mc], start=True, stop=True)
                nc.scalar.copy(
                    ot3[:, :, 64 * t:64 * t + mc],
                    pd[:, 0:4 * mc].rearrange("p (c m) -> p c m", m=mc))

            def body():
                for t in range(NT + 3):
                    if 2 <= t <= NT + 1:
                        post_b(t - 2)
                    if t < NT:
                        conv_tile(t)
                    if 1 <= t <= NT:
                        post_a(t - 1)
                    if t >= 3:
                        ds_tile(t - 3)
                    # flush output columns as their tiles finish downsampling
                    flush = {18: (0, 1024), 27: (1024, 512),
                             31: (1536, 256), NT + 2: (1792, 256)}
                    if t in flush and not os.environ.get("KM_NODMA"):
                        m0, mw = flush[t]
                        for c in range(NCHUNK):
                            nc.sync.dma_start(
                                out=o_out[128 * c:128 * (c + 1), m0:m0 + mw],
                                in_=out_sb[:, MOUT * c + m0:MOUT * c + m0 + mw])

            if repeat:
                with tc.For_i(0, repeat, 1) as _i:
                    body()
            else:
                body()

    nc.finalize()
    return nc


_CACHE = {}


def _get_program(repeat=0):
    if repeat not in _CACHE:
        _CACHE[repeat] = build_program(repeat)
    return _CACHE[repeat]


def make_in_maps(x, conv_w, conv_b, score_w):
    wv, scb, pcs, bias_bc = _host_consts(np.asarray(conv_w, np.float32),
                                         np.asarray(conv_b, np.float32),
                                         np.asarray(score_w, np.float32))
    xb = np.asarray(x, np.float32).astype(NPBF)
    return [{"x": np.ascontiguousarray(xb[b]), "wv": wv, "scb": scb,
             "pcs": pcs, "bias_bc": bias_bc} for b in range(B)]


def kernel(x, conv_w, conv_b, score_w):
    nc = _get_program()
    in_maps = make_in_maps(x, conv_w, conv_b, score_w)
    res = run_bass_kernel_spmd(nc, in_maps, core_ids=list(range(B)))
    return np.stack([res.results[b]["out"] for b in range(B)], axis=0)
